# revision 15
# baseline (speedup 1.0000x reference)
"""Trainium2 Bass kernel for nn_Binary_CNN2 (binarized CNN, eval mode).

Data-parallel over 8 NeuronCores: batch 4096 -> 512 per core.

Software-pipelined per-core body (steady state, per sub-body):
  - PE stream interleaves FC1 (reads a[cur], DoubleRow fp8) with conv
    matmuls (writes a[1-cur]) so the HAM clock gate stays warm through
    the shallow-K conv matmuls and the PE never idles on stage-0.
  - stage-0 for the next sub-body (x sign -> DMA-transpose -> xpad
    DRAM scatter) runs on DVE + HWDGE queues under the matmul stream.
  - im2col gathers (SWDGE, gpsimd) prefetch rhs slabs per lam; the
    xpad rewrite for the next sub-body is FIFO-ordered after this
    sub-body's im2col reads on the same gpsimd queue.
  - FC2 (w3-stationary matmuls + unstabilized log_softmax) at the end;
    out is stored as [C, B] (contiguous, few descriptors) and
    transposed on host.

Pipeline: prologue (s0 + conv -> a[0]); each sub-body then runs
fc1/fc2 on a[cur] while conv fills a[1-cur]. Every sub-body emits a
full correct output (inputs repeat), so the single-shot kernel is
prologue + one sub-body.
"""

import numpy as np
import ml_dtypes

import concourse.bass as bass
import concourse.mybir as mybir
import concourse.tile as tile
from concourse import bacc
from concourse.bass_utils import run_bass_kernel_spmd

EPS = 1e-5
NCORES = 8
B = 512          # batch per core
BH = 256         # batch half (conv matmul free dim)
H = 2048
C = 10
F32 = mybir.dt.float32
BF16 = mybir.dt.bfloat16
FP8 = mybir.dt.float8e4
UNROLL = 4

# conv row-groups over the 28 image rows: sizes 8,8,8,4 (pool-pair aligned)
# valid pooled-row-pair indices per group: g<3 -> ilp 0..3, g=3 -> ilp 0..1
NPART_FOR_ILP = [128, 128, 96, 96]  # FC1 contraction rows valid per ilp

SIMPLIFY = set()


def _f(c, k):
    """FC1 feature index map: chunk c=(ilp*14+jp), row k=(g*32+o) -> flat f."""
    ilp, jp = divmod(c, 14)
    g, o = divmod(k, 32)
    if g < 3:
        ip = 4 * g + ilp
    else:
        if ilp >= 2:
            return None
        ip = 12 + ilp
    return o * 196 + ip * 14 + jp


def build_nc(loop_n=None, parts=("s0", "conv", "fc1", "fc2"), simplify=None):
    nc = bacc.Bacc("TRN2", target_bir_lowering=False, debug=False,
                   num_devices=NCORES)

    xin = nc.dram_tensor("x", [128, 4 * 28 * 28], BF16, kind="ExternalInput")
    wc = nc.dram_tensor("wc", [128, 128], FP8, kind="ExternalInput")
    negt1 = nc.dram_tensor("negt1", [128, 1], F32, kind="ExternalInput")
    w2a = nc.dram_tensor("w2a", [16, 128, 28, 128], FP8, kind="ExternalInput")
    w2c = nc.dram_tensor("w2c", [16, 96, 28, 128], FP8, kind="ExternalInput")
    s2t = nc.dram_tensor("s2t", [128, 16], F32, kind="ExternalInput")
    t2t = nc.dram_tensor("t2t", [128, 16], F32, kind="ExternalInput")
    w3t = nc.dram_tensor("w3t", [16, 128, C], BF16, kind="ExternalInput")
    b3c = nc.dram_tensor("b3c", [C, 1], F32, kind="ExternalInput")
    out = nc.dram_tensor("out", [C, B], F32, kind="ExternalOutput")

    # padded transposed image: xpad[i' (34 incl 4 slack), j' (32), b] fp8
    xpad = nc.dram_tensor("xpad", [34 * 32 * B], FP8, kind="Internal")

    with tile.TileContext(nc) as tc:
        with (
            tc.tile_pool(name="consts", bufs=1) as consts,
            tc.tile_pool(name="persist", bufs=1) as persist,
        ):
            # ---- constants to SBUF (outside any timing loop) ----
            wc_sb = consts.tile([128, 128], FP8)
            nc.sync.dma_start(wc_sb[:], wc.ap())
            negt1_sb = consts.tile([128, 1], F32)
            nc.sync.dma_start(negt1_sb[:], negt1.ap())
            s2_sb = consts.tile([128, 16], F32)
            nc.sync.dma_start(s2_sb[:], s2t.ap())
            t2_sb = consts.tile([128, 16], F32)
            nc.sync.dma_start(t2_sb[:], t2t.ap())
            w3_sb = consts.tile([128, 16, C], BF16)
            nc.sync.dma_start(w3_sb[:], w3t.ap().rearrange("t p c -> p t c"))
            b3_sb = consts.tile([C, 1], F32)
            nc.sync.dma_start(b3_sb[:], b3c.ap())
            ones10 = consts.tile([C, C], BF16)
            nc.vector.memset(ones10[:], 1.0)

            # persistent state
            a_sb = persist.tile([128, 2, 4, 14, B], FP8)     # double-buffered
            zt_sb = persist.tile([128, 16, B], BF16)
            xb_sb = persist.tile([128, 4, 28, 32], BF16)
            nc.vector.memset(xb_sb[:], 0.0)
            xT_sb = persist.tile([128, 7, 4, 128], BF16)
            xT8_sb = persist.tile([128, 7, 4, 128], FP8)
            # explicit rings
            x_sb = [persist.tile([128, 4, 28 * 28], BF16, tag=f"x{i}", name=f"x{i}")
                    for i in range(2)]
            rhs_sb = [persist.tile([128, 2, 28, B], FP8, tag=f"rhs{i}", name=f"rhs{i}")
                      for i in range(2)]
            for t in rhs_sb:
                nc.vector.memset(t[:], 0.0)
            w2_sb = [persist.tile([128, 56, 128], FP8, tag=f"w2{i}", name=f"w2{i}")
                     for i in range(3)]
            out_t = [persist.tile([C, B], F32, tag=f"ot{i}", name=f"ot{i}") for i in range(2)]
            e_sb = [persist.tile([C, B], BF16, tag=f"e{i}", name=f"e{i}") for i in range(2)]
            lns_sb = [persist.tile([C, B], F32, tag=f"ln{i}", name=f"ln{i}") for i in range(2)]
            pm_sb = [persist.tile([128, BH], BF16, tag=f"pm{i}", name=f"pm{i}")
                     for i in range(3)]
            sq_sb = [persist.tile([128, 4, 3, BH], FP8, tag=f"sq{i}", name=f"sq{i}")
                     for i in range(2)]
            m1_sb = [persist.tile([128, 2, 3, BH], FP8, tag=f"m1{i}", name=f"m1{i}")
                     for i in range(2)]

            with tc.tile_pool(name="cpsum", bufs=1, space="PSUM") as cps:
                cq = [cps.tile([128, 4, BH], F32, tag=f"cq{i}", name=f"cq{i}")
                      for i in range(3)]
                zz = [cps.tile([128, B], F32, tag=f"zz{i}", name=f"zz{i}") for i in range(2)]
                # fc2 psum views aliased into conv psum banks (fc2 runs at
                # body end, after the conv drains of those banks)
                psl_ap = cq[0][0:C].rearrange("p s b -> p (s b)")[:, 0:B]
                sebc_ap = cq[1][0:C].rearrange("p s b -> p (s b)")[:, 0:B]

                # zero xpad borders once (rows 0 and 29..33 and j' wrap cols)
                with tc.tile_pool(name="ztmp0", bufs=1) as ztmp0:
                    zeros_sb = ztmp0.tile([128, 1088], FP8)
                    nc.vector.memset(zeros_sb[:], 0.0)
                    for q in range(4):
                        nc.gpsimd.dma_start(
                            bass.AP(xpad, q * 128 * 1088,
                                    [[1088, 128], [1, 1088]]),
                            zeros_sb[:])

                # ---------------- emission helpers ----------------
                def emit_xload(r):
                    nc.scalar.dma_start(
                        x_sb[r][:],
                        xin.ap().rearrange("p (bo f) -> p bo f", bo=4))

                def emit_sign(r):
                    # sign: (x >= 0) - 0.5 -> {+0.5, -0.5}; conv weights x2
                    nc.vector.tensor_scalar(
                        xb_sb[:, :, :, 0:28],
                        x_sb[r][:].rearrange("p bo (h w) -> p bo h w", h=28),
                        0.0, 0.5, mybir.AluOpType.is_ge,
                        mybir.AluOpType.subtract)

                def emit_transpose(k):
                    c, bo = divmod(k, 4)
                    src = xb_sb[:, bo].rearrange("p h w -> p (h w)")
                    nc.sync.dma_start(
                        xT_sb[:, c, bo, :],
                        src[:, c * 128:(c + 1) * 128],
                        transpose=True)

                def emit_xT_cast():
                    nc.vector.tensor_copy(xT8_sb[:], xT_sb[:])

                def emit_xpad_write():
                    # dst(q,c,bo,bl) = (c*128+q)*512 + 33*512 + bo*128 + bl
                    nc.sync.dma_start(
                        bass.AP(xpad, 33 * B,
                                [[B, 128], [128 * B, 7], [1, 512]]),
                        xT8_sb[:].rearrange("p c bo bl -> p c (bo bl)"))

                def emit_im2col(lam, slot):
                    rhs_t = rhs_sb[slot]
                    # one plain SWDGE fp8 gather per (dy,dx):
                    # [4 g-rows, 2 r-rows, 28*512 contiguous (j,b)]
                    for dy in range(3):
                        for dx in range(3):
                            p0 = dx * 12 + dy * 4
                            off = (2 * lam + dy) * 32 * B + dx * B
                            srcap = bass.AP(
                                xpad, off,
                                [[8 * 32 * B, 4], [32 * B, 2], [1, 28 * B]])
                            nc.sync.dma_start(rhs_t[p0:p0 + 4], srcap)

                def emit_w2_load(gc):
                    ht = gc % 16
                    w2 = w2_sb[gc % 3]
                    q = nc.sync if gc % 2 == 0 else nc.scalar
                    q.dma_start(w2[:, 0:28, :], w2a.ap()[ht])
                    q.dma_start(w2[0:96, 28:56, :], w2c.ap()[ht])

                def emit_conv_unit(u, slot, dst):
                    """4 conv matmuls (K=128 zero-padded) + pool/sign epilogue."""
                    lam, v = divmod(u % 112, 28)
                    bh, jp = divmod(v, 14)
                    kk = NPART_FOR_ILP[lam]
                    rhs_t = rhs_sb[slot]
                    psq = cq[u % 3]
                    for r in range(2):
                        for s in range(2):
                            nc.tensor.matmul(
                                psq[:, s * 2 + r, :],
                                wc_sb[:],
                                rhs_t[:, r, 2 * jp + s,
                                      bh * BH:(bh + 1) * BH],
                                start=True, stop=True)
                    bsl = slice(bh * BH, (bh + 1) * BH)
                    # jp<5: DVE-led drain; jp>=5: ACT-led drain with DVE
                    # pair-maxes batched per 3 consecutive jp
                    if jp < 5:
                        a_slice = a_sb[0:kk, dst, lam, jp, bsl]
                        pm = pm_sb[u % 3]
                        nc.vector.tensor_reduce(
                            pm[0:kk],
                            psq[0:kk].rearrange("p s b -> p b s"),
                            axis=mybir.AxisListType.X,
                            op=mybir.AluOpType.max)
                        nc.scalar.activation(
                            a_slice, pm[0:kk],
                            mybir.ActivationFunctionType.Sign,
                            bias=negt1_sb[0:kk])
                    else:
                        blk = ((lam * 2 + bh) * 3 + (jp - 5) // 3) % 2
                        bi = (jp - 5) % 3
                        sq = sq_sb[blk]
                        nc.scalar.activation(
                            sq[0:kk, :, bi, :], psq[0:kk],
                            mybir.ActivationFunctionType.Sign,
                            bias=negt1_sb[0:kk])
                        if bi == 2:
                            m1 = m1_sb[blk]
                            jp0 = jp - 2
                            nc.vector.tensor_tensor(
                                m1[0:kk, 0], sq[0:kk, 0], sq[0:kk, 1],
                                mybir.AluOpType.max)
                            nc.vector.tensor_tensor(
                                m1[0:kk, 1], sq[0:kk, 2], sq[0:kk, 3],
                                mybir.AluOpType.max)
                            nc.vector.tensor_tensor(
                                a_sb[0:kk, dst, lam, jp0:jp0 + 3, bsl],
                                m1[0:kk, 0], m1[0:kk, 1],
                                mybir.AluOpType.max)

                def emit_fc1_group(gc, g, src):
                    """7 DoubleRow matmuls: cp = g*7 .. g*7+6."""
                    psz = zz[gc % 2]
                    w2 = w2_sb[gc % 3]
                    for cp in range(g * 7, g * 7 + 7):
                        lam, jph = divmod(cp, 7)
                        jp = 2 * jph
                        c = lam * 14 + jp
                        kk = NPART_FOR_ILP[lam]
                        nc.tensor.matmul(
                            psz[:],
                            w2[0:kk, c:c + 2, :],
                            a_sb[0:kk, src, lam, jp:jp + 2, :],
                            start=(cp == 0), stop=(cp == 27),
                            perf_mode=mybir.MatmulPerfMode.DoubleRow)

                def emit_fc1_tail(gc):
                    ht = gc % 16
                    psz = zz[gc % 2]
                    nc.scalar.activation(
                        zt_sb[:, ht, :], psz[:],
                        mybir.ActivationFunctionType.Identity,
                        bias=t2_sb[:, ht:ht + 1],
                        scale=s2_sb[:, ht:ht + 1])
                    nc.vector.tensor_scalar(
                        zt_sb[:, ht, :], zt_sb[:, ht, :],
                        1.0, -1.0, mybir.AluOpType.min, mybir.AluOpType.max)

                def emit_fc2(r):
                    # logits.T [10, 512]; unstabilized log_softmax
                    for ht in range(16):
                        nc.tensor.matmul(psl_ap, w3_sb[:, ht, :],
                                         zt_sb[:, ht, :],
                                         start=(ht == 0), stop=(ht == 15))
                    e = e_sb[r]
                    nc.scalar.activation(
                        e[:], psl_ap, mybir.ActivationFunctionType.Exp,
                        bias=b3_sb[:])
                    nc.tensor.matmul(sebc_ap, ones10[:], e[:],
                                     start=True, stop=True)
                    lns = lns_sb[r]
                    nc.scalar.activation(lns[:], sebc_ap,
                                         mybir.ActivationFunctionType.Ln)
                    nc.vector.scalar_tensor_tensor(
                        out_t[r][:], psl_ap, b3_sb[:], lns[:],
                        mybir.AluOpType.add, mybir.AluOpType.subtract)
                    nc.sync.dma_start(out.ap(), out_t[r][:])

                # ---------------- phases ----------------
                state = {"u": 0, "piece": 0}

                def prologue():
                    emit_xload(0)
                    emit_sign(0)
                    for k in range(28):
                        emit_transpose(k)
                    emit_xT_cast()
                    emit_xpad_write()
                    for lam in range(4):
                        emit_im2col(lam, state["piece"] % 2)
                        state["piece"] += 1
                        for v in range(28):
                            u = state["u"]
                            emit_conv_unit(u, (u // 28) % 2, 0)
                            state["u"] += 1
                    emit_im2col(0, state["piece"] % 2)
                    state["piece"] += 1

                W2PF = 2   # w2 prefetch depth (chunks)

                def mega_body(subs):
                    """subs: list of `cur` values, one per pipelined sub-body.

                    Per sub-body: fc1/fc2 on a[cur]; conv -> a[1-cur]; s0 for
                    the next sub-body overlapped on DVE/DMA queues."""
                    total = len(subs) * 16
                    for k in range(W2PF):
                        emit_w2_load(k)
                    for gc in range(total):
                        sub, ht = divmod(gc, 16)
                        cur = subs[sub]
                        if gc + W2PF < total:
                            emit_w2_load(gc + W2PF)
                        # stage-0 emission schedule (local to sub-body)
                        if ht == 0:
                            emit_xload(cur)
                            emit_sign(cur)
                        elif ht in (2, 6, 10):
                            emit_im2col((ht + 2) // 4, state["piece"] % 2)
                            state["piece"] += 1
                        elif ht == 14:
                            emit_xT_cast()
                            emit_xpad_write()
                        for g in range(4):
                            emit_fc1_group(gc, g, cur)
                            nu = 2 if g < 3 else 1
                            for _ in range(nu):
                                u = state["u"]
                                emit_conv_unit(u, (u // 28) % 2, 1 - cur)
                                state["u"] += 1
                            if 2 <= ht <= 8:
                                emit_transpose(((ht - 2) * 4 + g) % 28)
                        emit_fc1_tail(gc)
                        if ht == 15:
                            emit_fc2(cur)
                            emit_im2col(0, state["piece"] % 2)
                            state["piece"] += 1

                prologue()
                if loop_n is None:
                    mega_body([0])
                else:
                    assert loop_n % UNROLL == 0
                    with tc.For_i(0, loop_n // UNROLL, 1):
                        mega_body([s % 2 for s in range(UNROLL)])

    nc.finalize()
    return nc


_NC_CACHE = {}


def _get_nc(loop_n=None, parts=("s0", "conv", "fc1", "fc2")):
    key = (loop_n, tuple(parts), tuple(sorted(SIMPLIFY)))
    if key not in _NC_CACHE:
        _NC_CACHE[key] = build_nc(loop_n, parts)
    return _NC_CACHE[key]


def _host_prep(W1, b1, g1, be1, m1, v1, W2, b2, g2, be2, m2, v2, W3, b3):
    """Precompute small device-side constant tensors (numpy, f32)."""
    s1 = (g1 / np.sqrt(v1 + EPS)).astype(np.float32)
    assert np.all(s1 != 0)
    # bn1 >= 0  <=>  sign(conv_nb - t1[o]) == sign(s1[o]); fold sign(s1)
    # into W2's columns so the device only computes sign(conv_nb - t1)
    t1 = (m1 - be1 / s1 - b1).astype(np.float32)
    sgn1 = np.where(s1 >= 0, 1.0, -1.0).astype(np.float32)
    negt1 = np.repeat(-t1[None, :], 4, axis=0).reshape(128, 1)

    wc = np.zeros((128, 128), np.float32)
    w1s = np.where(W1[:, 0] >= 0, 2.0, -2.0).astype(np.float32)  # [32,3,3] x2
    for dy in range(3):
        for dx in range(3):
            for g in range(4):
                p = dx * 12 + dy * 4 + g
                wc[p, g * 32:(g + 1) * 32] = w1s[:, dy, dx]
    wc = wc.astype(ml_dtypes.float8_e4m3)

    w2s = np.where(W2 >= 0, 1.0, -1.0).astype(np.float32)  # [H, F1]
    w2s = w2s * sgn1[np.arange(w2s.shape[1]) // 196][None, :]
    w2bp = np.zeros((16, 128, 56, 128), np.float32)  # [ht, k, c, hh]
    for c in range(56):
        ilp, jp = divmod(c, 14)
        for g in range(4):
            if _f(c, g * 32) is None:
                continue
            ip = 4 * g + ilp if g < 3 else 12 + ilp
            fs = np.arange(32) * 196 + ip * 14 + jp  # f for o=0..31
            # w2bp[ht, g*32+o, c, hh] = w2s[ht*128+hh, fs[o]]
            blk = w2s[:, fs].reshape(16, 128, 32)   # [ht, hh, o]
            w2bp[:, g * 32:(g + 1) * 32, c, :] = blk.transpose(0, 2, 1)
    w2bp = w2bp.astype(ml_dtypes.float8_e4m3)
    w2a = np.ascontiguousarray(w2bp[:, :, 0:28, :])
    w2c = np.ascontiguousarray(w2bp[:, 0:96, 28:56, :])

    s2 = (g2 / np.sqrt(v2 + EPS)).astype(np.float32)
    t2 = (be2 + s2 * (b2 - m2)).astype(np.float32)
    s2t = s2.reshape(16, 128).T.copy()
    t2t = t2.reshape(16, 128).T.copy()

    w3t = np.ascontiguousarray(W3.T.astype(ml_dtypes.bfloat16)).reshape(16, 128, C)
    b3c = np.ascontiguousarray(b3.astype(np.float32).reshape(C, 1))
    return dict(wc=wc, negt1=negt1, w2a=w2a, w2c=w2c, s2t=s2t, t2t=t2t,
                w3t=w3t, b3c=b3c)


def _make_in_maps(x, consts):
    xs = np.asarray(x, np.float32).reshape(NCORES, B, 28 * 28)
    in_maps = []
    for i in range(NCORES):
        # [b, f] -> [p=b%128, bo=b//128, f] contiguous bf16
        xi = xs[i].reshape(4, 128, 28 * 28).transpose(1, 0, 2)
        xi = np.ascontiguousarray(xi).astype(ml_dtypes.bfloat16)
        m = {"x": xi.reshape(128, 4 * 28 * 28)}
        m.update(consts)
        in_maps.append(m)
    return in_maps


def _prep_all(inputs):
    names = ["W1", "b1", "g1", "be1", "m1", "v1", "W2", "b2", "g2", "be2",
             "m2", "v2", "W3", "b3"]
    return _host_prep(*[np.asarray(inputs[n], np.float32) for n in names])


def kernel(x, **weights):
    consts = _prep_all(weights)
    nc = _get_nc(None)
    in_maps = _make_in_maps(x, consts)
    res = run_bass_kernel_spmd(nc, in_maps, core_ids=list(range(NCORES)))
    outs = [np.ascontiguousarray(res.results[i]["out"].T)
            for i in range(NCORES)]
    return np.concatenate(outs, axis=0).astype(np.float32)


def _make_runner(nc, in_maps):
    """Build a reusable executor with inputs resident on device (no re-upload)."""
    import jax
    import jax.numpy as jnp
    from jax.sharding import Mesh, PartitionSpec, NamedSharding
    from jax.experimental.shard_map import shard_map
    from concourse import bass2jax
    from concourse.bass2jax import _bass_exec_p, install_neuronx_cc_hook

    install_neuronx_cc_hook()
    n_cores = len(in_maps)
    partition_name = nc.partition_id_tensor.name if nc.partition_id_tensor else None
    in_names, out_names, out_avals, zero_outs = [], [], [], []
    for alloc in nc.m.functions[0].allocations:
        if not isinstance(alloc, mybir.MemoryLocationSet):
            continue
        name = alloc.memorylocations[0].name
        if alloc.kind == "ExternalInput":
            if name != partition_name:
                in_names.append(name)
        elif alloc.kind == "ExternalOutput":
            shape = tuple(alloc.tensor_shape)
            dtype = mybir.dt.np(alloc.dtype)
            out_names.append(name)
            out_avals.append(jax.core.ShapedArray(shape, dtype))
            zero_outs.append(np.zeros(shape, dtype))
    n_params = len(in_names)
    n_outs = len(out_avals)
    in_names.extend(out_names)
    if partition_name is not None:
        in_names.append(partition_name)
    donate = tuple(range(n_params, n_params + n_outs))

    def _body(*args):
        operands = list(args)
        if partition_name is not None:
            operands.append(bass2jax.partition_id_tensor())
        outs = _bass_exec_p.bind(
            *operands, out_avals=tuple(out_avals), in_names=tuple(in_names),
            out_names=tuple(out_names), lowering_input_output_aliases=(),
            sim_require_finite=True, sim_require_nnan=True, nc=nc)
        return tuple(outs)

    devices = jax.devices()[:n_cores]
    mesh = Mesh(np.asarray(devices), ("core",))
    sharded = jax.jit(
        shard_map(_body, mesh=mesh,
                  in_specs=(PartitionSpec("core"),) * (n_params + n_outs),
                  out_specs=(PartitionSpec("core"),) * n_outs,
                  check_rep=False),
        donate_argnums=donate, keep_unused=True)
    shard = NamedSharding(mesh, PartitionSpec("core"))
    per_core = [[np.asarray(m[nm]) for nm in in_names[:n_params]]
                for m in in_maps]
    dev_in = [jax.device_put(
                np.concatenate([per_core[c][i] for c in range(n_cores)],
                               axis=0), shard)
              for i in range(n_params)]
    concat_zero_shapes = [((n_cores * z.shape[0],) + z.shape[1:], z.dtype)
                          for z in zero_outs]

    def run():
        zeros = [jnp.zeros(s, d, device=shard) for s, d in concat_zero_shapes]
        outs = sharded(*dev_in, *zeros)
        jax.block_until_ready(outs)
        return outs

    return run


def measure_exec_ns(inputs, n_lo=4, n_hi=132, reps=11):
    """HW exec time per pipeline iteration via looped-kernel wall-clock delta."""
    import time
    consts = _prep_all(inputs)
    in_maps = _make_in_maps(inputs["x"], consts)

    def med_time(loop_n):
        nc = _get_nc(loop_n, measure_exec_ns.parts)
        run = _make_runner(nc, in_maps)
        run()  # compile + warm
        ts = []
        for _ in range(reps):
            t0 = time.time()
            run()
            ts.append(time.time() - t0)
        ts.sort()
        return ts[len(ts) // 2], ts

    t_lo, all_lo = med_time(n_lo)
    t_hi, all_hi = med_time(n_hi)
    measure_exec_ns.last = (all_lo, all_hi)
    return (t_hi - t_lo) / (n_hi - n_lo) * 1e9


measure_exec_ns.parts = ("s0", "conv", "fc1", "fc2")
build_nc_looped = build_nc  # marker for test.py
